# revision 29
# baseline (speedup 1.0000x reference)
"""Trainium2 Bass kernel for bidirectional cross-attention (nn_CrossAttention).

Reference computation (per batch b, N=1024 tokens, D=768 dims):
    sim1  = image1 @ image2^T            [N, N]
    out2  = l2norm(softmax(sim1) @ image2) + 2*image2
    sim2  = image2 @ image1^T
    out1  = l2norm(softmax(sim2) @ image1) + 2*image1

Key algebraic simplification: l2norm(softmax(S) @ V) == l2norm(exp(S - rowmax) @ V)
because the softmax denominator is a positive per-row scalar that the L2
normalization cancels.  So the kernel never computes the softmax sum.

Sharding: pure data parallel, B=16 batches -> 2 per core across 8 cores.

v2 (fp8): both GEMMs run in fp8e4 (E4M3) with MatmulPerfMode.DoubleRow
(0.5 cycles/row, two 128-contraction subtiles per instruction).  The exp
output P is stored directly in fp8 (entries are in (0, 1] after exact
rowmax subtraction, so E4M3 loses only weights < 2^-9 whose softmax
contribution is negligible).  V is the fp8 image itself.  The epilogue
avoids Sqrt (which lives in a different ACT table set than Exp and forced
~1.5us table reloads) via 1/||O|| = exp(-0.5*ln(sum(O^2))): the whole
kernel only uses {Exp, Ln, Square, Copy} = one table set, zero reloads.

Per-core pipeline (matmuls fp8, accumulation + epilogue in fp32):
  - SWDGE cast-DMA loads images as fp8 natural chunk tiles [128, 8, 768]
  - PE transposes (identity matmul) build the [D, N] layout [128, 6, 1024]
  - mm1: S[q,:] = 3 DoubleRow matmuls x 2 PSUM banks (contract 2x128/instr)
  - softmax: -rowmax on DVE, exp via ACT (PSUM -> fp8 SBUF)
  - PE transposes P -> P^T [128, 8, 128] fp8 (PSUM), DVE copies to SBUF
  - mm2: O = P^T.T @ V, 4 DoubleRow matmuls x 2 banks
  - epilogue: ss = sum(O^2) (ACT Square+accum), inv = exp(-0.5*ln(ss))
              (ACT Ln + Exp), T1 = O*inv (ACT Copy scale=inv),
              T3 = T1 + 2*resid (gpsimd), store

The three PE stages are software-pipelined (mm1(i) | ptrans(i-1) | mm2(i-2))
so the PE never waits on the softmax chain of the same iteration.
"""

import os
import sys

import numpy as np

for _p in ("/opt/trn_rl_repo", "/root/.axon_site/_ro/trn_rl_repo"):
    if os.path.isdir(_p) and _p not in sys.path:
        sys.path.append(_p)

B, N, D = 16, 1024, 768
NCORES = 8
BPC = B // NCORES  # batches per core
P = 128
NT = N // P  # 8 token chunks
DT = D // P  # 6 feature chunks

_PROGRAM_CACHE = {}


def build_program():
    """Build the per-core Bass program (SPMD: identical on all cores)."""
    import concourse.mybir as mybir
    import concourse.tile as tile
    from concourse import bacc
    from concourse.masks import make_identity

    f32 = mybir.dt.float32
    bf16 = mybir.dt.bfloat16
    f8 = mybir.dt.float8e4
    AF = mybir.ActivationFunctionType
    ALU = mybir.AluOpType
    AX = mybir.AxisListType
    DR = mybir.MatmulPerfMode.DoubleRow

    # Bacc (not plain Bass): its compile() pass splits multi-semaphore waits
    # into event-semaphore sequences — TRN2 instructions encode only 1 wait.
    nc = bacc.Bacc(None)

    # All activation funcs this kernel uses (Exp, Ln, Square, Copy, Identity)
    # coexist in the 'natural_log_exp_and_others' table set, but the table
    # placement pass resolves each func to the first set containing it, which
    # bounces between sets and costs ~1.3us per ACT_TABLE_LOAD.  Restrict the
    # (cached, shared) table map so these funcs only appear in the one set:
    # indices of the sets are untouched, so the emitted act_func_set_id still
    # correctly indexes act_info.json.
    from concourse.hw_specs import get_activation_tables

    _tabs = get_activation_tables(nc.m.arch)
    _keep = "natural_log_exp_and_others"
    if _keep in _tabs:
        _ours = {
            mybir.ActivationFunctionType.Exp,
            mybir.ActivationFunctionType.Ln,
            mybir.ActivationFunctionType.Square,
            mybir.ActivationFunctionType.Copy,
            mybir.ActivationFunctionType.Identity,
        }
        assert _ours <= _tabs[_keep]
        for _name, _s in _tabs.items():
            if _name != _keep:
                _s -= _ours
    img_dram = {
        1: nc.declare_dram_parameter("image1", [BPC, N, D], f32, isOutput=False),
        2: nc.declare_dram_parameter("image2", [BPC, N, D], f32, isOutput=False),
    }
    out_dram = {
        1: nc.declare_dram_parameter("out1", [BPC, N, D], f32, isOutput=True),
        2: nc.declare_dram_parameter("out2", [BPC, N, D], f32, isOutput=True),
    }

    with tile.TileContext(nc) as tc:
        with (
            tc.tile_pool(name="const", bufs=1) as const_pool,
            tc.tile_pool(name="imgs", bufs=2) as imgs_pool,
            tc.tile_pool(name="work", bufs=4) as work,
            tc.tile_pool(name="outs", bufs=6) as outs,
            tc.tile_pool(name="stats", bufs=8) as stats,
            tc.tile_pool(name="spsum", bufs=3, space="PSUM") as spsum,
            tc.tile_pool(name="tpsum", bufs=2, space="PSUM") as tpsum,
        ):
            ident = const_pool.tile([P, P], bf16)
            make_identity(nc, ident[:])

            imgb = {}   # (b, im) -> [P, NT, D] natural bf16 chunk tile
            img8 = {}   # (b, im) -> [P, NT, D] natural fp8 chunk tile (mm2 V)
            imgT = {}   # (b, im) -> [P, DT, N] transposed fp8 tile

            def prep_loads(b):
                """Issue image loads for batch b. img1 via SWDGE cast-DMA
                (f32->bf16), img2 via HWDGE f32 + ACT cast (parallel paths).
                fp8 copies of the natural chunks (mm2's V) cast on gpsimd."""
                for im in (1, 2):
                    nb = imgs_pool.tile([P, NT, D], bf16, tag=f"imgb{im}", name=f"imgb{im}")
                    n8 = imgs_pool.tile([P, NT, D], f8, tag=f"img8{im}", name=f"img8{im}")
                    # bf16 per-chunk (fine-grained deps: transposes start as
                    # soon as each chunk lands); fp8 V as one batched DMA
                    # (only needed by mm2, much later)
                    for kc in range(NT):
                        nc.gpsimd.dma_start(
                            nb[:, kc, :], img_dram[im][b, kc * P : (kc + 1) * P, :]
                        )
                    src_r = img_dram[im][b].rearrange("(nt p) d -> p nt d", p=P)
                    nc.gpsimd.dma_start(n8[:], src_r)
                    imgb[(b, im)] = nb
                    img8[(b, im)] = n8

            def prep_groups(b):
                """Return 12 closures, each PE-transposing one (im, dc) group.
                Transposes run in bf16 (fp8 PE transpose needs strided out);
                the PSUM->SBUF copy converts to fp8."""
                tbs = {}
                for im in (1, 2):
                    tbs[im] = imgs_pool.tile([P, DT, N], f8, tag=f"imgT{im}", name=f"imgT{im}")
                    imgT[(b, im)] = tbs[im]

                def make(im, dc):
                    def g():
                        nb = imgb[(b, im)]
                        tp = tpsum.tile([P, NT, P], bf16, tag="tp")
                        for kc in range(NT):
                            nc.tensor.transpose(
                                tp[:, kc, :],
                                nb[:, kc, dc * P : (dc + 1) * P],
                                ident[:],
                            )
                        nc.scalar.activation(tbs[im][:, dc, :], tp[:], AF.Copy)
                    return g

                return [make(im, dc) for dc in range(DT) for im in (1, 2)]

            # iteration = (batch, q_img, kv_img, q_tile); dir1 out2, dir2 out1
            iters = []
            for b in range(BPC):
                for qi in range(NT):
                    iters.append((b, 1, 2, qi))
                    iters.append((b, 2, 1, qi))
            n = len(iters)
            n0 = n // BPC  # iterations per batch

            state = {}

            def stage_a(it):
                """mm1 (fp8 DoubleRow) + softmax issue (rowmax DVE, exp ACT)."""
                b, q_im, kv_im, qi = it
                S = spsum.tile([P, N], f32, tag="S")
                qT = imgT[(b, q_im)]
                kT = imgT[(b, kv_im)]
                for dp in range(DT // 2):
                    lhsT = qT[:, 2 * dp : 2 * dp + 2, qi * P : (qi + 1) * P]
                    nc.tensor.matmul(
                        S[:, :512], lhsT, kT[:, 2 * dp : 2 * dp + 2, :512],
                        start=(dp == 0), stop=(dp == DT // 2 - 1), perf_mode=DR,
                    )
                    nc.tensor.matmul(
                        S[:, 512:], lhsT, kT[:, 2 * dp : 2 * dp + 2, 512:],
                        start=(dp == 0), stop=(dp == DT // 2 - 1), perf_mode=DR,
                    )
                negmax = stats.tile([P, 1], f32, tag="negmax")
                nc.vector.tensor_reduce(
                    negmax, S[:], axis=AX.X, op=ALU.max, negate=True
                )
                Pw = work.tile([P, N], bf16, tag="P")
                nc.scalar.activation(Pw, S[:], AF.Exp, bias=negmax, scale=1.0)
                state[("P", it)] = Pw
                # after exp has read S, the S PSUM region is dead: mm2 reuses
                # its first 768 columns as O (saves banks -> spsum bufs=3,
                # decoupling mm2(i) from the epilogue drain of O(i-1))
                state[("S", it)] = S
                # prefetch the residual tile 2 slots ahead of stage_b and
                # pre-double it (gpsimd, off the critical path)
                # resid2 = 2*img built by DMA alone: write img, then
                # accumulate img once more (SWDGE accum into SBUF) — no
                # compute engine touches it.
                src = img_dram[kv_im][b, qi * P : (qi + 1) * P, :]
                resid2 = work.tile([P, D], f32, tag="resid2")
                nc.sync.dma_start(resid2[:], src)
                nc.gpsimd.dma_start(resid2[:], src, accum_op=ALU.add)
                state[("R", it)] = resid2

            def stage_t(it):
                """PE-transpose P -> P^T (fp8), evacuate to SBUF via DVE."""
                Pw = state.pop(("P", it))
                tp = tpsum.tile([P, NT, P], bf16, tag="tp")
                PT = work.tile([P, NT, P], f8, tag="PT")
                for kc in range(NT):
                    nc.tensor.transpose(
                        tp[:, kc, :], Pw[:, kc * P : (kc + 1) * P], ident[:]
                    )
                    if kc == NT // 2 - 1:
                        # evacuate the first half early: mm2's first DoubleRow
                        # pairs only need PT[:, :4, :], so it can start while
                        # the PE still transposes the second half
                        nc.vector.tensor_copy(PT[:, : NT // 2, :], tp[:, : NT // 2, :])
                nc.vector.tensor_copy(PT[:, NT // 2 :, :], tp[:, NT // 2 :, :])
                state[("PT", it)] = PT

            def stage_b(it):
                """mm2 (fp8 DoubleRow) + normalize + residual + store."""
                b, q_im, kv_im, qi = it
                PT = state.pop(("PT", it))
                V = img8[(b, kv_im)]
                O = state.pop(("S", it))[:, :D]
                for kp in range(NT // 2):
                    lhsT = PT[:, 2 * kp : 2 * kp + 2, :]
                    nc.tensor.matmul(
                        O[:, :512], lhsT, V[:, 2 * kp : 2 * kp + 2, :512],
                        start=(kp == 0), stop=(kp == NT // 2 - 1), perf_mode=DR,
                    )
                    nc.tensor.matmul(
                        O[:, 512:], lhsT, V[:, 2 * kp : 2 * kp + 2, 512:],
                        start=(kp == 0), stop=(kp == NT // 2 - 1), perf_mode=DR,
                    )
                # epilogue: out = l2norm(O) + 2*img_kv
                #   ss  = sum(O^2)            (ACT Square + accum)
                #   inv = exp(-0.5*ln(ss))    (ACT Ln, ACT Exp — same table set)
                #   T1  = O*inv               (ACT Copy with scale AP; frees O)
                #   T3  = T1 + 2*resid        (gpsimd tensor_tensor, SBUF only)
                sq = work.tile([P, D], f32, tag="sq")
                ss = stats.tile([P, 1], f32, tag="ss")
                nc.scalar.activation(sq, O[:], AF.Square, accum_out=ss)
                lss = stats.tile([P, 1], f32, tag="lss")
                nc.scalar.activation(lss, ss, AF.Ln)
                inv = stats.tile([P, 1], f32, tag="inv")
                nc.scalar.activation(inv, lss, AF.Exp, scale=-0.5)
                resid2 = state.pop(("R", it))
                T3 = outs.tile([P, D], f32, tag="T3")
                nc.vector.scalar_tensor_tensor(
                    out=T3, in0=O[:], scalar=inv, in1=resid2[:],
                    op0=ALU.mult, op1=ALU.add,
                )
                nc.sync.dma_start(
                    out_dram[kv_im][b, qi * P : (qi + 1) * P, :], T3[:]
                )

            # batch-0 prep up front; batch b+1 loads issued 8 iters before the
            # batch boundary and its PE transposes injected into the pipeline
            # tail of batch b, where the PE would otherwise stall.
            prep_loads(0)
            for g in prep_groups(0):
                g()
            pending_groups = []
            for gi in range(n + 2):
                # stage_b first: its epilogue frees the single O PSUM buffer
                # early instead of queueing behind rowmax/PTcopy
                if gi >= 2:
                    stage_b(iters[gi - 2])
                if gi < n:
                    stage_a(iters[gi])
                bidx = gi // n0 + 1  # next batch index
                if gi % n0 == n0 - 11 and bidx < BPC:
                    prep_loads(bidx)
                if gi % n0 == n0 - 6 and bidx < BPC:
                    pending_groups = prep_groups(bidx)
                if 1 <= gi <= n:
                    stage_t(iters[gi - 1])
                if pending_groups:
                    for g in pending_groups[:4]:
                        g()
                    pending_groups = pending_groups[4:]

    return nc


def _get_program():
    if "nc" not in _PROGRAM_CACHE:
        nc = build_program()
        if not nc.is_finalized():
            nc.finalize()
        _PROGRAM_CACHE["nc"] = nc
    return _PROGRAM_CACHE["nc"]


def kernel(image1: np.ndarray, image2: np.ndarray):
    from concourse.bass_utils import run_bass_kernel_spmd

    image1 = np.ascontiguousarray(image1, dtype=np.float32)
    image2 = np.ascontiguousarray(image2, dtype=np.float32)
    assert image1.shape == (B, N, D) and image2.shape == (B, N, D)

    nc = _get_program()
    core_ids = list(range(NCORES))
    in_maps = [
        {
            "image1": image1[c * BPC : (c + 1) * BPC],
            "image2": image2[c * BPC : (c + 1) * BPC],
        }
        for c in core_ids
    ]
    res = run_bass_kernel_spmd(nc, in_maps, core_ids)
    out1 = np.concatenate([res.results[c]["out1"] for c in core_ids], axis=0)
    out2 = np.concatenate([res.results[c]["out2"] for c in core_ids], axis=0)
    return out1, out2
